# revision 39
# baseline (speedup 1.0000x reference)
"""Single-head causal self-attention on 8 NeuronCores (data-parallel over batch).

Reference computation (per batch element b):
    Q = X @ Wq + bq; K = X @ Wk + bk; V = X @ Wv + bv        # [T, DK]
    S = Q @ K.T / sqrt(DK)  (causal masked)
    out = softmax(S) @ V                                      # [T, DK]

Device strategy (one batch element per core), bf16 (PSUM accum fp32):
  - Host passes X.T tiles and packed weights in bf16 (halves HBM traffic
    and SBUF pressure vs fp32).
  - Pass A stationary packs [Wk | Wv] per 128-row C-chunk, so the psum
    holds K.T in partitions 0:64 and V.T in partitions 64:128. Pass B is
    just Wq -> Q.T in a [64, T] psum (no duplication).
  - Inbound DMA is descriptor-generation bound (~3-4us per 128-row
    transfer, serialized per queue, ~9us queue startup), so transfer
    COUNT is minimized: all weights/constants/biases ride in one packed
    [128, 1732] bf16 "wall" tensor (biases bitcast to bf16 pairs), and
    each queue's first transfer is on the k=0 critical path (wall on
    scalar, x0 on sync, x1 on gpsimd); remaining X tiles stripe in k
    order.
  - k=0..6 accumulate; the k=7 matmuls are emitted per 512-column chunk
    with that chunk's psum->SBUF drains right behind them, alternating
    scalar/vector (bias added exactly: Identity activation with a
    per-partition bias vector / tensor_scalar_add).
  - V.T tiles are PE-transposed (4 per psum tile, in a scoped pool
    between projections and attention) into [s, dk] stationaries with a
    ones column appended; the ones column makes the output matmul also
    produce the softmax denominator l (output row 64).
  - Scores are computed transposed, S.T[s, t] = K.T^T @ Q.T, trimmed to
    the causal boundary at 128-column granularity; exp runs on scalar
    (scale=1/8 fused) reading psum fp32, writing bf16; causality = one
    triangular mask multiply per diagonal block (gpsimd, SBUF-only) +
    trimmed matmuls (no memsets needed).
  - The P@V contraction accumulates the four 512-wide output chunks in
    psum; per-chunk drain (vector, bf16) + output DMA (sync) as soon as
    each chunk's accumulation ends. Output rides as bf16 ([65, T]); the
    host upcasts and normalizes.
  - PE stream is software-pipelined two stages deep: scores(i+1) and
    scores(i+2) are emitted before PV(i), so exp on scalar has two score
    stages of matmul time to complete before PV consumes its output
    (et pool bufs=3 holds exactly i, i+1, i+2).
  - Device output per core: [65, T] = rows 0:64 unnormalized O.T, row 64
    l. Host computes (O_unnorm / l).T.
"""

import sys

sys.path.insert(0, "/opt/trn_rl_repo")

import numpy as np
import ml_dtypes

B, T, C, DK = 8, 2048, 1024, 64
KT = C // 128          # 8 k-tiles in the contraction over C
NS = T // 128          # 16 s-tiles (key blocks)
NCHUNK = T // 512      # 4 output chunks of 512
SCALE = 1.0 / np.sqrt(DK)
BF16 = ml_dtypes.bfloat16

_CACHE = {}


def _build():
    from concourse import bass, bacc, tile

    mybir = bass.mybir
    f32 = mybir.dt.float32
    bf16 = mybir.dt.bfloat16

    nc = bacc.Bacc(
        "TRN2", target_bir_lowering=False, debug=False, num_devices=B
    )

    xt_d = nc.dram_tensor("xt", [KT, 128, T], bf16, kind="ExternalInput")
    # one packed block: wkv | wq | tri | ident | bkv | bq (biases bitcast
    # to bf16 pairs) -- a single 128-row DMA, since descriptor generation
    # (~3-4us per 128-row transfer, serialized per queue) dominates the
    # inbound critical path, not bytes
    WALL = KT * 128 + KT * 64 + 192 + 4
    wall_d = nc.dram_tensor("wall", [128, WALL], bf16, kind="ExternalInput")
    out_d = nc.dram_tensor("out", [65, T], bf16, kind="ExternalOutput")

    EXP = mybir.ActivationFunctionType.Exp
    IDENT = mybir.ActivationFunctionType.Identity

    with tile.TileContext(nc) as tc:
        with tc.tile_pool(name="const", bufs=1) as cpool, \
             tc.tile_pool(name="weights", bufs=1) as wpool, \
             tc.tile_pool(name="x", bufs=1) as xpool, \
             tc.tile_pool(name="acts", bufs=1) as apool:

            wall = wpool.tile([128, WALL], bf16)
            tri = wall[:, 1536:1664]
            ident64 = wall[64:128, 1664:1728]
            bkv = wall[:, 1728:1730].bitcast(f32)
            bq = wall[0:64, 1730:1732].bitcast(f32)

            def wkv_s(k):
                return wall[:, 128 * k:128 * (k + 1)]

            def wq_s(k):
                return wall[:, 1024 + 64 * k:1024 + 64 * (k + 1)]

            xts = []
            for k in range(KT):
                xk = xpool.tile([128, T], bf16, tag=f"x{k}", name=f"x{k}")
                xts.append(xk)

            def x_s(k, sl):
                return xts[k][:, sl]

            # one transfer per queue up front: wall on scalar (fastest),
            # x0 whole on sync, x1 on gpsimd; rest striped in k order
            nc.scalar.dma_start(out=wall[:], in_=wall_d[:])
            nc.sync.dma_start(out=xts[0][:], in_=xt_d[0])
            nc.gpsimd.dma_start(out=xts[1][:], in_=xt_d[1])
            xdma = {2: nc.scalar, 3: nc.sync, 4: nc.gpsimd,
                    5: nc.scalar, 6: nc.sync, 7: nc.gpsimd}
            for k in range(2, KT):
                xdma[k].dma_start(out=xts[k][:], in_=xt_d[k])

            # persistent activations
            vk = apool.tile([128, T], bf16, tag="vk")   # K.T rows 0:64, V.T rows 64:128
            qq = apool.tile([64, T], bf16, tag="qq")    # Q.T
            v1 = apool.tile([128, NS * 65], bf16, tag="v1")  # [V_i | 1] stationaries
            osb = apool.tile([65, T], bf16, tag="osb")

            nc.gpsimd.memset(v1[:], 1.0)

            # ---------------- projections ----------------
            with tc.tile_pool(name="pproj", bufs=1, space="PSUM") as pproj:
                # per-chunk psum tiles: tile-granular dependency tracking
                # would otherwise stall the k=7 matmul of chunk c+1 on the
                # drain (a reader) of chunk c
                psAc, psBc = [], []
                for c in range(NCHUNK):
                    pa = pproj.tile([128, 512], f32, tag=f"psA{c}", name=f"psA{c}")
                    pb = pproj.tile([64, 512], f32, tag=f"psB{c}", name=f"psB{c}")
                    psAc.append(pa)
                    psBc.append(pb)
                for k in range(KT - 1):
                    for c in range(NCHUNK):
                        sl = slice(512 * c, 512 * (c + 1))
                        nc.tensor.matmul(
                            psAc[c][:], wkv_s(k), x_s(k, sl),
                            start=(k == 0), stop=False,
                        )
                    for c in range(NCHUNK):
                        sl = slice(512 * c, 512 * (c + 1))
                        nc.tensor.matmul(
                            psBc[c][:], wq_s(k), x_s(k, sl),
                            start=(k == 0), stop=False,
                        )
                # k=7 per chunk; drains right behind each chunk, interleaved
                # so vector delivers vk c0 first (gates transposes) and
                # scalar delivers qq c0 first (gates scores(0))
                k = KT - 1
                for c in range(NCHUNK):
                    sl = slice(512 * c, 512 * (c + 1))
                    nc.tensor.matmul(
                        psAc[c][:], wkv_s(k), x_s(k, sl),
                        start=False, stop=True,
                    )
                    nc.tensor.matmul(
                        psBc[c][:], wq_s(k), x_s(k, sl),
                        start=False, stop=True,
                    )
                    if c % 2 == 0:
                        nc.scalar.activation(
                            qq[:, sl], psBc[c][:], IDENT, bias=bq[:]
                        )
                        nc.vector.tensor_scalar_add(vk[:, sl], psAc[c][:], bkv[:])
                    else:
                        nc.vector.tensor_scalar_add(qq[:, sl], psBc[c][:], bq[:])
                        nc.scalar.activation(
                            vk[:, sl], psAc[c][:], IDENT, bias=bkv[:]
                        )

            # ---------------- V transposes ----------------
            # scoped psum pool between projections and attention so the
            # attention pools get the full 8 banks
            with tc.tile_pool(name="pv", bufs=2, space="PSUM") as pv:
                for g in range(4):
                    vt = pv.tile([128, 256], bf16, tag="vt")
                    for c in range(4):
                        i = 4 * g + c
                        nc.tensor.transpose(
                            vt[:, 64 * c:64 * (c + 1)],
                            vk[64:128, 128 * i:128 * (i + 1)], ident64[:],
                        )
                    for c in range(4):
                        i = 4 * g + c
                        nc.vector.tensor_copy(
                            v1[:, 65 * i:65 * i + 64], vt[:, 64 * c:64 * (c + 1)]
                        )

            # ---------------- attention ----------------
            with tc.tile_pool(name="po", bufs=1, space="PSUM") as po, \
                 tc.tile_pool(name="pst", bufs=2, space="PSUM") as pst, \
                 tc.tile_pool(name="et", bufs=3) as etpool:

                ops = [
                    po.tile([65, 512], f32, tag=f"o{j}", name=f"o{j}")
                    for j in range(NCHUNK)
                ]

                ets = [None] * NS

                def emit_scores(i):
                    # S.T[s, t] for t in [ts, 2048), trimmed to causal boundary
                    ts = 128 * i
                    et = etpool.tile([128, T], bf16, tag="et")
                    ets[i] = et
                    for tb in range(ts // 1024, 2):
                        st = pst.tile([128, 1024], f32, tag="st")
                        for cc in range(2):
                            t0 = 1024 * tb + 512 * cc
                            lo = max(t0, ts)
                            if t0 + 512 <= lo:
                                continue
                            nc.tensor.matmul(
                                st[:, lo - 1024 * tb:t0 + 512 - 1024 * tb],
                                vk[0:64, 128 * i:128 * (i + 1)],
                                qq[:, lo:t0 + 512],
                                start=True, stop=True,
                            )
                        off = max(0, ts - 1024 * tb)
                        nc.scalar.activation(
                            et[:, 1024 * tb + off:1024 * (tb + 1)],
                            st[:, off:1024],
                            EXP, scale=SCALE,
                        )
                    # causal mask on the diagonal 128-block (gpsimd: SBUF-only)
                    nc.gpsimd.tensor_mul(
                        et[:, ts:ts + 128], et[:, ts:ts + 128], tri[:]
                    )

                def emit_pv(i):
                    ts = 128 * i
                    jmin = i // 4
                    et = ets[i]
                    for j in range(jmin, NCHUNK):
                        lo = max(512 * j, ts)
                        nc.tensor.matmul(
                            ops[j][:, lo - 512 * j:512],
                            v1[:, 65 * i:65 * i + 65],
                            et[:, lo:512 * (j + 1)],
                            start=(i == 0), stop=(i == 4 * j + 3),
                        )
                    # drain any output chunk whose accumulation just finished
                    for j in range(jmin, NCHUNK):
                        if i == 4 * j + 3:
                            sl = slice(512 * j, 512 * (j + 1))
                            nc.vector.tensor_copy(osb[:, sl], ops[j][:])
                            nc.sync.dma_start(out=out_d[:, sl], in_=osb[:, sl])

                emit_scores(0)
                emit_scores(1)
                for i in range(NS):
                    if i + 2 < NS:
                        emit_scores(i + 2)
                    emit_pv(i)

    nc.compile()
    return nc


def _get_nc():
    if "nc" not in _CACHE:
        _CACHE["nc"] = _build()
    return _CACHE["nc"]


def make_in_maps(X, Wq, bq, Wk, bk, Wv, bv):
    X = np.asarray(X, dtype=np.float32)
    Wq = np.asarray(Wq, dtype=np.float32)
    Wk = np.asarray(Wk, dtype=np.float32)
    Wv = np.asarray(Wv, dtype=np.float32)
    bq = np.asarray(bq, dtype=np.float32)
    bk = np.asarray(bk, dtype=np.float32)
    bv = np.asarray(bv, dtype=np.float32)

    wkv = np.ascontiguousarray(
        np.concatenate([Wk, Wv], axis=1).reshape(KT, 128, 128)
        .transpose(1, 0, 2).reshape(128, KT * 128)
    ).astype(BF16)
    wq = np.ascontiguousarray(
        Wq.reshape(KT, 128, 64).transpose(1, 0, 2).reshape(128, KT * 64)
    ).astype(BF16)

    wall = np.zeros((128, KT * 128 + KT * 64 + 192 + 4), dtype=BF16)
    wall[:, 0:1024] = wkv
    wall[:, 1024:1536] = wq
    wall[:, 1536:1664] = np.triu(np.ones((128, 128), dtype=np.float32)).astype(BF16)
    wall[64:128, 1664:1728] = np.eye(64, dtype=np.float32).astype(BF16)
    wall[:, 1728:1730] = (
        np.concatenate([bk, bv]).reshape(128, 1).astype(np.float32).view(BF16)
    )
    wall[0:64, 1730:1732] = bq.reshape(64, 1).astype(np.float32).view(BF16)

    in_maps = []
    for b in range(B):
        xt = np.ascontiguousarray(X[b].T).reshape(KT, 128, T).astype(BF16)
        in_maps.append({"xt": xt, "wall": wall})
    return in_maps


def kernel(X, Wq, bq, Wk, bk, Wv, bv):
    from concourse.bass_utils import run_bass_kernel_spmd

    nc = _get_nc()
    in_maps = make_in_maps(X, Wq, bq, Wk, bk, Wv, bv)
    res = run_bass_kernel_spmd(nc, in_maps, list(range(B)))

    out = np.empty((B, T, DK), dtype=np.float32)
    for b in range(B):
        r = res.results[b]["out"].astype(np.float32)
        out[b] = (r[:64] / r[64:65]).T
    return out
